# revision 51
# baseline (speedup 1.0000x reference)
import sys

sys.path.insert(0, "/opt/trn_rl_repo")

import numpy as np
import ml_dtypes

NCORES = 8
B, FULL_N, D = 4, 2048, 1024
NH = 16
DK = 64  # head dim
HPC = NH // NCORES  # heads per core = 2
CW = HPC * DK  # output columns per core = 128
DC = D // 128  # D chunks = 8

_CACHE = {}
LAST_RESULTS = None


def _build(n_rows):
    """SPMD Bass program for one core. Each core computes batch-0 attention
    for its 2 heads (the reference only uses att[0]) and adds it to its
    column slice of tgt for all batches.

    All-bf16 datapath: tgt[0]/memory[0]/weights arrive host-transposed and
    pre-packed in bf16 (halves HBM traffic vs fp32, and bf16 keeps the PE
    at full HAM duty for ~90us where fp32r/fp8-DoubleRow configs get
    duty-cycle throttled). Scores are computed transposed (k on partitions)
    so softmax's P feeds P.T@V with no P transposes. exp() runs on ScalarE
    with scale 1/sqrt(dk) and bias -4 (cancels exactly in the softmax
    ratio), writing P as bf16; V carries an appended ones column so the PV
    accumulation also yields the softmax row sums. Projections/scores/PV
    are software-pipelined across 512-row q-groups (scores of qg+1 overlap
    PV of qg) so neither PE nor ScalarE ever starves; input group DMAs are
    triple-buffered so all loads stream from t=0."""
    import concourse.mybir as mybir
    import concourse.tile as tile
    from concourse import bacc
    from concourse.masks import make_identity

    fp32 = mybir.dt.float32
    bf16 = mybir.dt.bfloat16
    fp8 = mybir.dt.float8e4

    RT = n_rows // 128  # row tiles
    G = n_rows // 512  # 512-row groups
    QG = G
    KC = RT  # k chunks of 128

    nc = bacc.Bacc(None, target_bir_lowering=False)
    # host pre-arranged layouts (fully linear per-partition DMA):
    #   mem0g/tgt0g[g, p, d, n] = x[g*512 + n, d*128 + p]   (x = memory[0]/tgt[0])
    #   w*[p, d, q]             = W[c*CW + q, d*128 + p]     (core c's slice)
    #   tgtc[b, p, t, c]        = tgt[b, t*128 + p, c0 + c]
    mem0g = nc.declare_dram_parameter("mem0g", [G, 128, DC, 512], bf16, isOutput=False)
    tgt0g = nc.declare_dram_parameter("tgt0g", [G, 128, DC, 512], bf16, isOutput=False)
    wqt = nc.declare_dram_parameter("wqt", [128, DC, CW], bf16, isOutput=False)
    wkt = nc.declare_dram_parameter("wkt", [128, DC, CW], bf16, isOutput=False)
    wvt = nc.declare_dram_parameter("wvt", [128, DC, CW], bf16, isOutput=False)
    tgtc = nc.declare_dram_parameter("tgtc", [B, 128, RT, CW], bf16, isOutput=False)
    outc = nc.declare_dram_parameter("outc", [B, 128, RT, CW], bf16, isOutput=True)

    Exp = mybir.ActivationFunctionType.Exp
    DR = mybir.MatmulPerfMode.DoubleRow
    scale = 1.0 / np.sqrt(DK)
    EBIAS = -4.0  # exp(s*scale + EBIAS): keeps p in fp8e4m3 range; cancels in ratio

    with tile.TileContext(nc) as tc:
        # single SBUF + single PSUM pool (tags carry the per-buffer cycling):
        # every tile pool costs an all-engine drain round at release, ~8us of
        # teardown barriers with 13 pools
        with (
            tc.tile_pool(name="sbp", bufs=1) as const,
            tc.tile_pool(name="psp", bufs=2, space="PSUM") as psp,
        ):
            persist = const
            ident = const.tile([128, 128], bf16, tag="ident")
            make_identity(nc, ident)
            ebias = const.tile([128, 1], fp32, tag="ebias")
            nc.vector.memset(ebias, EBIAS)

            KT_gs = [
                persist.tile([128, 512], bf16, tag=f"KT{g}", name=f"KT{g}")
                for g in range(G)
            ]
            QT_gs = [
                persist.tile([128, 512], bf16, tag=f"QT{g}", name=f"QT{g}")
                for g in range(G)
            ]
            # Vp[g][p, h, t, :] = V chunk (4g + t) for head h:
            # rows = k-positions, cols = dk dims + ones column
            VW = DK + 1
            Vp_gs = [
                persist.tile(
                    [128, HPC, 4, VW], bf16, tag=f"Vp{g}", name=f"Vp{g}"
                )
                for g in range(G)
            ]
            for g in range(G):
                nc.vector.memset(Vp_gs[g][:, :, :, DK], 1.0)
            tgtc_sb = persist.tile([128, B, RT, CW], bf16, tag="tgtc")

            wst_pool = mgrp_pool = tgrp_pool = vt_pool = const
            usb_pool = att_pool = small_pool = pt_pool = const
            ps_pj = ps_sc = ps_pv = psp
            if True:
                # PE warmup during the initial DMA wait (p-state ramp)
                for _ in range(16):
                    pw = ps_pj.tile([128, 128], bf16, tag="pj", name="warm")
                    nc.tensor.transpose(pw, ident, ident)

                WTs = {}
                for name, w in (("k", wkt), ("v", wvt), ("q", wqt)):
                    wt = wst_pool.tile([128, DC, CW], bf16, tag=f"wt{name}")
                    nc.sync.dma_start(out=wt, in_=w[:, :, :])
                    WTs[name] = wt

                def emit_K(g):
                    memg = mgrp_pool.tile(
                        [128, DC, 512], bf16, tag="mg", bufs=3, name=f"memg{g}"
                    )
                    for d in range(DC):
                        nc.sync.dma_start(out=memg[:, d, :], in_=mem0g[g, :, d, :])
                    pk = ps_pj.tile([128, 512], fp32, tag="pj", name=f"psk{g}")
                    for d in range(DC):
                        nc.tensor.matmul(
                            pk, WTs["k"][:, d, :], memg[:, d, :],
                            start=(d == 0), stop=(d == DC - 1),
                        )
                    nc.vector.tensor_copy(out=KT_gs[g], in_=pk)
                    return memg

                def emit_V(g, memg):
                    pv = ps_pj.tile([128, 512], fp32, tag="pj", name=f"psv{g}")
                    for d in range(DC):
                        nc.tensor.matmul(
                            pv, WTs["v"][:, d, :], memg[:, d, :],
                            start=(d == 0), stop=(d == DC - 1),
                        )
                    vt_g = vt_pool.tile([128, 512], bf16, tag="vtg", bufs=2, name=f"vt{g}")
                    nc.vector.tensor_copy(out=vt_g, in_=pv)
                    for t in range(4):
                        ptb = ps_pj.tile([128, 128], bf16, tag="pj", name=f"vtr{g}_{t}")
                        nc.tensor.transpose(
                            ptb, vt_g[:, t * 128 : (t + 1) * 128], ident
                        )
                        for h in range(HPC):
                            nc.vector.tensor_copy(
                                out=Vp_gs[g][:, h, t, 0:DK],
                                in_=ptb[:, h * DK : (h + 1) * DK],
                            )

                def emit_tgt_group(g):
                    tgtg = tgrp_pool.tile(
                        [128, DC, 512], bf16, tag="tg", bufs=3, name=f"tgtg{g}"
                    )
                    for d in range(DC):
                        nc.sync.dma_start(out=tgtg[:, d, :], in_=tgt0g[g, :, d, :])
                    pq = ps_pj.tile([128, 512], fp32, tag="pj", name=f"psq{g}")
                    for d in range(DC):
                        nc.tensor.matmul(
                            pq, WTs["q"][:, d, :], tgtg[:, d, :],
                            start=(d == 0), stop=(d == DC - 1),
                        )
                    nc.vector.tensor_copy(out=QT_gs[g], in_=pq)

                def emit_st_block(qg, pts, jp):
                    # one score+exp pair: k-chunks 2*jp, 2*jp+1 for both heads
                    psts = [
                        ps_sc.tile(
                            [128, 2, 512], fp32, tag="st", name=f"st{qg}_{jp}_{h}"
                        )
                        for h in range(HPC)
                    ]
                    for jj in range(2):
                        j = jp * 2 + jj
                        kg, kt = j // 4, j % 4
                        for h in range(HPC):
                            hs = h * DK
                            nc.tensor.matmul(
                                psts[h][:, jj, :],
                                KT_gs[kg][hs : hs + DK, kt * 128 : (kt + 1) * 128],
                                QT_gs[qg][hs : hs + DK, :],
                                start=True, stop=True,
                            )
                    for h in range(HPC):
                        nc.scalar.activation(
                            out=pts[h][:, jp * 2 : jp * 2 + 2, :],
                            in_=psts[h],
                            func=Exp,
                            scale=float(scale),
                            bias=ebias,
                        )

                def new_pts(qg):
                    return [
                        pt_pool.tile(
                            [128, KC, 512], bf16, tag=f"pt{h}", bufs=2,
                            name=f"pt{h}_{qg}",
                        )
                        for h in range(HPC)
                    ]

                all_pts = {}
                # ---- Phase A: K0 -> Q0 -> first scores BEFORE any V work, so
                # ScalarE's exp stream (83% dense once started, and it paces
                # the whole back half of the kernel) starts ~8us earlier; V
                # projections and transposes backfill between score blocks.
                all_pts[0] = new_pts(0)
                memgs = {}
                memgs[0] = emit_K(0)
                emit_tgt_group(0)
                for jp in (0, 1):
                    emit_st_block(0, all_pts[0], jp)
                emit_V(0, memgs[0])
                memgs[1] = emit_K(1)
                for jp in (2, 3):
                    emit_st_block(0, all_pts[0], jp)
                emit_V(1, memgs[1])
                emit_tgt_group(1)
                for g in range(2, G):
                    memgs[g] = emit_K(g)
                    for jp in range(2 * g, 2 * g + 2):
                        emit_st_block(0, all_pts[0], jp)
                    emit_V(g, memgs[g])
                # residual input, only needed for the adds at each q-group tail
                for b in range(B):
                    nc.sync.dma_start(out=tgtc_sb[:, b, :, :], in_=tgtc[b])

                # ---- Phase B: per q-group: scores(qg+1) overlap PV(qg) ----
                for qg in range(QG):
                    if qg + 2 < QG:
                        emit_tgt_group(qg + 2)
                    if qg + 1 < QG:
                        all_pts[qg + 1] = new_pts(qg + 1)
                        for jp in range(KC // 2):
                            emit_st_block(qg + 1, all_pts[qg + 1], jp)
                    pts = all_pts[qg]
                    att_t = att_pool.tile(
                        [128, 4, CW], bf16, tag="att", bufs=2, name=f"att{qg}"
                    )
                    for h in range(HPC):
                        hs = h * DK
                        pu = ps_pv.tile([VW, 512], fp32, tag="u", name=f"u{qg}{h}")
                        for j in range(KC):
                            nc.tensor.matmul(
                                pu,
                                Vp_gs[j // 4][:, h, j % 4, :],
                                pts[h][:, j, :],
                                start=(j == 0), stop=(j == KC - 1),
                            )
                        pu_sb = usb_pool.tile([DK + 1, 512], bf16, tag="usb", bufs=2)
                        nc.vector.tensor_copy(out=pu_sb, in_=pu[0 : DK + 1, :])
                        for s in range(4):
                            patb = ps_pj.tile(
                                [128, DK + 1], bf16, tag="pj", name=f"atr{qg}{h}{s}"
                            )
                            nc.tensor.transpose(
                                patb,
                                pu_sb[:, s * 128 : (s + 1) * 128],
                                ident[0 : DK + 1, 0 : DK + 1],
                            )
                            rec = small_pool.tile([128, 1], fp32, tag="rec", bufs=8)
                            nc.vector.reciprocal(rec, patb[:, DK : DK + 1])
                            nc.vector.tensor_scalar_mul(
                                att_t[:, s, hs : hs + DK],
                                in0=patb[:, 0:DK],
                                scalar1=rec,
                            )
                    # broadcast add + store for this q-group's rows (one DMA
                    # per batch: every attempt to split these across more
                    # queues lost more to per-DMA issue cost than it gained
                    # in transfer parallelism)
                    for b in range(B):
                        nc.vector.tensor_add(
                            out=tgtc_sb[:, b, qg * 4 : (qg + 1) * 4, :],
                            in0=tgtc_sb[:, b, qg * 4 : (qg + 1) * 4, :],
                            in1=att_t,
                        )
                        nc.sync.dma_start(
                            out=outc[b, :, qg * 4 : (qg + 1) * 4, :],
                            in_=tgtc_sb[:, b, qg * 4 : (qg + 1) * 4, :],
                        )

    nc.finalize()
    return nc


def _get_nc(n_rows):
    if n_rows not in _CACHE:
        _CACHE[n_rows] = _build(n_rows)
    return _CACHE[n_rows]


def _bf16(x):
    return np.ascontiguousarray(x, dtype=np.float32).astype(ml_dtypes.bfloat16)


def _run(tgt, memory, Wq, Wk, Wv, trace=False):
    global LAST_RESULTS
    from concourse.bass_utils import run_bass_kernel_spmd

    n_rows = tgt.shape[1]
    G = n_rows // 512
    RT = n_rows // 128
    nc = _get_nc(n_rows)

    tgt = np.ascontiguousarray(tgt, dtype=np.float32)
    memory = np.ascontiguousarray(memory, dtype=np.float32)

    def grouped(x0):  # [N, D] -> [G, 128, DC, 512] with [g,p,d,n] = x0[g*512+n, d*128+p]
        return _bf16(
            np.ascontiguousarray(
                x0.reshape(G, 512, DC, 128).transpose(0, 3, 2, 1)
            )
        )

    mem0g = grouped(memory[0])
    tgt0g = grouped(tgt[0])

    def wslice(W, sl):  # [D_out, D_in] slice -> [128, DC, CW]
        return _bf16(
            np.ascontiguousarray(
                W[sl, :].T.reshape(DC, 128, CW).transpose(1, 0, 2)
            )
        )

    in_maps = []
    for c in range(NCORES):
        sl = slice(c * CW, (c + 1) * CW)
        tc_arr = _bf16(
            np.ascontiguousarray(
                tgt[:, :, sl].reshape(B, RT, 128, CW).transpose(0, 2, 1, 3)
            )
        )
        in_maps.append(
            {
                "mem0g": mem0g,
                "tgt0g": tgt0g,
                "wqt": wslice(Wq, sl),
                "wkt": wslice(Wk, sl),
                "wvt": wslice(Wv, sl),
                "tgtc": tc_arr,
            }
        )
    res = run_bass_kernel_spmd(nc, in_maps, list(range(NCORES)), trace=trace)
    LAST_RESULTS = res
    out = np.empty((B, n_rows, NCORES * CW), dtype=np.float32)
    for c in range(NCORES):
        oc = np.asarray(res.results[c]["outc"], dtype=np.float32)
        # [B, 128, RT, CW] -> [B, N, CW]
        out[:, :, c * CW : (c + 1) * CW] = oc.transpose(0, 2, 1, 3).reshape(
            B, n_rows, CW
        )
    return out


def kernel(tgt, memory, Wq, Wk, Wv):
    return _run(tgt, memory, Wq, Wk, Wv)


# revision 52
# speedup vs baseline: 1.0255x; 1.0255x over previous
import sys

sys.path.insert(0, "/opt/trn_rl_repo")

import numpy as np
import ml_dtypes

NCORES = 8
B, FULL_N, D = 4, 2048, 1024
NH = 16
DK = 64  # head dim
HPC = NH // NCORES  # heads per core = 2
CW = HPC * DK  # output columns per core = 128
DC = D // 128  # D chunks = 8

_CACHE = {}
LAST_RESULTS = None


def _build(n_rows):
    """SPMD Bass program for one core. Each core computes batch-0 attention
    for its 2 heads (the reference only uses att[0]) and adds it to its
    column slice of tgt for all batches.

    All-bf16 datapath: tgt[0]/memory[0]/weights arrive host-transposed and
    pre-packed in bf16 (halves HBM traffic vs fp32, and bf16 keeps the PE
    at full HAM duty for ~90us where fp32r/fp8-DoubleRow configs get
    duty-cycle throttled). Scores are computed transposed (k on partitions)
    so softmax's P feeds P.T@V with no P transposes. exp() runs on ScalarE
    with scale 1/sqrt(dk) and bias -4 (cancels exactly in the softmax
    ratio), writing P as bf16; V carries an appended ones column so the PV
    accumulation also yields the softmax row sums. Projections/scores/PV
    are software-pipelined across 512-row q-groups (scores of qg+1 overlap
    PV of qg) so neither PE nor ScalarE ever starves; input group DMAs are
    triple-buffered so all loads stream from t=0."""
    import concourse.mybir as mybir
    import concourse.tile as tile
    from concourse import bacc
    from concourse.masks import make_identity

    fp32 = mybir.dt.float32
    bf16 = mybir.dt.bfloat16
    fp8 = mybir.dt.float8e4

    RT = n_rows // 128  # row tiles
    G = n_rows // 512  # 512-row groups
    QG = G
    KC = RT  # k chunks of 128

    nc = bacc.Bacc(None, target_bir_lowering=False)
    # host pre-arranged layouts (fully linear per-partition DMA):
    #   mem0g/tgt0g[g, p, d, n] = x[g*512 + n, d*128 + p]   (x = memory[0]/tgt[0])
    #   w*[p, d, q]             = W[c*CW + q, d*128 + p]     (core c's slice)
    #   tgtc[b, p, t, c]        = tgt[b, t*128 + p, c0 + c]
    mem0g = nc.declare_dram_parameter("mem0g", [G, 128, DC, 512], bf16, isOutput=False)
    tgt0g = nc.declare_dram_parameter("tgt0g", [G, 128, DC, 512], bf16, isOutput=False)
    wqt = nc.declare_dram_parameter("wqt", [128, DC, CW], bf16, isOutput=False)
    wkt = nc.declare_dram_parameter("wkt", [128, DC, CW], bf16, isOutput=False)
    wvt = nc.declare_dram_parameter("wvt", [128, DC, CW], bf16, isOutput=False)
    tgtc = nc.declare_dram_parameter("tgtc", [B, 128, RT, CW], bf16, isOutput=False)
    outc = nc.declare_dram_parameter("outc", [B, 128, RT, CW], bf16, isOutput=True)

    Exp = mybir.ActivationFunctionType.Exp
    DR = mybir.MatmulPerfMode.DoubleRow
    scale = 1.0 / np.sqrt(DK)
    EBIAS = -4.0  # exp(s*scale + EBIAS): keeps p in fp8e4m3 range; cancels in ratio

    with tile.TileContext(nc) as tc:
        # single SBUF + single PSUM pool (tags carry the per-buffer cycling):
        # every tile pool costs an all-engine drain round at release, ~8us of
        # teardown barriers with 13 pools
        with (
            tc.tile_pool(name="sbp", bufs=1) as const,
            tc.tile_pool(name="psp", bufs=2, space="PSUM") as psp,
        ):
            persist = const
            ident = const.tile([128, 128], bf16, tag="ident")
            make_identity(nc, ident)
            ebias = const.tile([128, 1], fp32, tag="ebias")
            nc.vector.memset(ebias, EBIAS)

            KT_gs = [
                persist.tile([128, 512], bf16, tag=f"KT{g}", name=f"KT{g}")
                for g in range(G)
            ]
            QT_gs = [
                persist.tile([128, 512], bf16, tag=f"QT{g}", name=f"QT{g}")
                for g in range(G)
            ]
            # Vp[g][p, h, t, :] = V chunk (4g + t) for head h:
            # rows = k-positions, cols = dk dims + ones column
            VW = DK + 1
            Vp_gs = [
                persist.tile(
                    [128, HPC, 4, VW], bf16, tag=f"Vp{g}", name=f"Vp{g}"
                )
                for g in range(G)
            ]
            for g in range(G):
                nc.vector.memset(Vp_gs[g][:, :, :, DK], 1.0)
            tgtc_sb = persist.tile([128, B, RT, CW], bf16, tag="tgtc")

            wst_pool = mgrp_pool = tgrp_pool = vt_pool = const
            usb_pool = att_pool = small_pool = pt_pool = const
            ps_pj = ps_sc = ps_pv = psp
            if True:
                # PE warmup during the initial DMA wait (p-state ramp)
                for _ in range(16):
                    pw = ps_pj.tile([128, 128], bf16, tag="pj", name="warm")
                    nc.tensor.transpose(pw, ident, ident)

                WTs = {}
                for name, w in (("k", wkt), ("v", wvt), ("q", wqt)):
                    wt = wst_pool.tile([128, DC, CW], bf16, tag=f"wt{name}")
                    nc.sync.dma_start(out=wt, in_=w[:, :, :])
                    WTs[name] = wt

                def emit_K(g):
                    memg = mgrp_pool.tile(
                        [128, DC, 512], bf16, tag="mg", bufs=3, name=f"memg{g}"
                    )
                    for d in range(DC):
                        nc.sync.dma_start(out=memg[:, d, :], in_=mem0g[g, :, d, :])
                    pk = ps_pj.tile([128, 512], fp32, tag="pj", name=f"psk{g}")
                    for d in range(DC):
                        nc.tensor.matmul(
                            pk, WTs["k"][:, d, :], memg[:, d, :],
                            start=(d == 0), stop=(d == DC - 1),
                        )
                    nc.vector.tensor_copy(out=KT_gs[g], in_=pk)
                    return memg

                def emit_V(g, memg):
                    pv = ps_pj.tile([128, 512], fp32, tag="pj", name=f"psv{g}")
                    for d in range(DC):
                        nc.tensor.matmul(
                            pv, WTs["v"][:, d, :], memg[:, d, :],
                            start=(d == 0), stop=(d == DC - 1),
                        )
                    vt_g = vt_pool.tile([128, 512], bf16, tag="vtg", bufs=2, name=f"vt{g}")
                    nc.vector.tensor_copy(out=vt_g, in_=pv)
                    for t in range(4):
                        ptb = ps_pj.tile([128, 128], bf16, tag="pj", name=f"vtr{g}_{t}")
                        nc.tensor.transpose(
                            ptb, vt_g[:, t * 128 : (t + 1) * 128], ident
                        )
                        for h in range(HPC):
                            nc.vector.tensor_copy(
                                out=Vp_gs[g][:, h, t, 0:DK],
                                in_=ptb[:, h * DK : (h + 1) * DK],
                            )

                def emit_tgt_group(g):
                    tgtg = tgrp_pool.tile(
                        [128, DC, 512], bf16, tag="tg", bufs=3, name=f"tgtg{g}"
                    )
                    for d in range(DC):
                        nc.sync.dma_start(out=tgtg[:, d, :], in_=tgt0g[g, :, d, :])
                    pq = ps_pj.tile([128, 512], fp32, tag="pj", name=f"psq{g}")
                    for d in range(DC):
                        nc.tensor.matmul(
                            pq, WTs["q"][:, d, :], tgtg[:, d, :],
                            start=(d == 0), stop=(d == DC - 1),
                        )
                    nc.vector.tensor_copy(out=QT_gs[g], in_=pq)

                def emit_st_block(qg, pts, jp):
                    # one score+exp pair: k-chunks 2*jp, 2*jp+1 for both heads
                    psts = [
                        ps_sc.tile(
                            [128, 2, 512], fp32, tag="st", name=f"st{qg}_{jp}_{h}"
                        )
                        for h in range(HPC)
                    ]
                    for jj in range(2):
                        j = jp * 2 + jj
                        kg, kt = j // 4, j % 4
                        for h in range(HPC):
                            hs = h * DK
                            nc.tensor.matmul(
                                psts[h][:, jj, :],
                                KT_gs[kg][hs : hs + DK, kt * 128 : (kt + 1) * 128],
                                QT_gs[qg][hs : hs + DK, :],
                                start=True, stop=True,
                            )
                    for h in range(HPC):
                        nc.scalar.activation(
                            out=pts[h][:, jp * 2 : jp * 2 + 2, :],
                            in_=psts[h],
                            func=Exp,
                            scale=float(scale),
                            bias=ebias,
                        )

                def new_pts(qg):
                    return [
                        pt_pool.tile(
                            [128, KC, 512], bf16, tag=f"pt{h}", bufs=2,
                            name=f"pt{h}_{qg}",
                        )
                        for h in range(HPC)
                    ]

                all_pts = {}
                # ---- Phase A: K0 -> Q0 -> first scores BEFORE any V work, so
                # ScalarE's exp stream (83% dense once started, and it paces
                # the whole back half of the kernel) starts ~8us earlier; V
                # projections and transposes backfill between score blocks.
                all_pts[0] = new_pts(0)
                memgs = {}
                # group 0 special-cased: mem/tgt chunk DMAs interleave on the
                # SP issue queue so Q-proj's inputs land alongside K-proj's
                # instead of ~7us later
                memg0 = mgrp_pool.tile(
                    [128, DC, 512], bf16, tag="mg", bufs=3, name="memg0"
                )
                tgtg0 = tgrp_pool.tile(
                    [128, DC, 512], bf16, tag="tg", bufs=3, name="tgtg0"
                )
                for d in range(DC):
                    nc.sync.dma_start(out=memg0[:, d, :], in_=mem0g[0, :, d, :])
                    nc.sync.dma_start(out=tgtg0[:, d, :], in_=tgt0g[0, :, d, :])
                pk0 = ps_pj.tile([128, 512], fp32, tag="pj", name="psk0")
                for d in range(DC):
                    nc.tensor.matmul(
                        pk0, WTs["k"][:, d, :], memg0[:, d, :],
                        start=(d == 0), stop=(d == DC - 1),
                    )
                nc.vector.tensor_copy(out=KT_gs[0], in_=pk0)
                pq0 = ps_pj.tile([128, 512], fp32, tag="pj", name="psq0")
                for d in range(DC):
                    nc.tensor.matmul(
                        pq0, WTs["q"][:, d, :], tgtg0[:, d, :],
                        start=(d == 0), stop=(d == DC - 1),
                    )
                nc.vector.tensor_copy(out=QT_gs[0], in_=pq0)
                memgs[0] = memg0
                for jp in (0, 1):
                    emit_st_block(0, all_pts[0], jp)
                emit_V(0, memgs[0])
                memgs[1] = emit_K(1)
                for jp in (2, 3):
                    emit_st_block(0, all_pts[0], jp)
                emit_V(1, memgs[1])
                emit_tgt_group(1)
                for g in range(2, G):
                    memgs[g] = emit_K(g)
                    for jp in range(2 * g, 2 * g + 2):
                        emit_st_block(0, all_pts[0], jp)
                    emit_V(g, memgs[g])
                # residual input, only needed for the adds at each q-group tail
                for b in range(B):
                    nc.sync.dma_start(out=tgtc_sb[:, b, :, :], in_=tgtc[b])

                # ---- Phase B: per q-group: scores(qg+1) overlap PV(qg) ----
                for qg in range(QG):
                    if qg + 2 < QG:
                        emit_tgt_group(qg + 2)
                    if qg + 1 < QG:
                        all_pts[qg + 1] = new_pts(qg + 1)
                        for jp in range(KC // 2):
                            emit_st_block(qg + 1, all_pts[qg + 1], jp)
                    pts = all_pts[qg]
                    att_t = att_pool.tile(
                        [128, 4, CW], bf16, tag="att", bufs=2, name=f"att{qg}"
                    )
                    for h in range(HPC):
                        hs = h * DK
                        pu = ps_pv.tile([VW, 512], fp32, tag="u", name=f"u{qg}{h}")
                        for j in range(KC):
                            nc.tensor.matmul(
                                pu,
                                Vp_gs[j // 4][:, h, j % 4, :],
                                pts[h][:, j, :],
                                start=(j == 0), stop=(j == KC - 1),
                            )
                        pu_sb = usb_pool.tile([DK + 1, 512], bf16, tag="usb", bufs=2)
                        nc.vector.tensor_copy(out=pu_sb, in_=pu[0 : DK + 1, :])
                        for s in range(4):
                            patb = ps_pj.tile(
                                [128, DK + 1], bf16, tag="pj", name=f"atr{qg}{h}{s}"
                            )
                            nc.tensor.transpose(
                                patb,
                                pu_sb[:, s * 128 : (s + 1) * 128],
                                ident[0 : DK + 1, 0 : DK + 1],
                            )
                            rec = small_pool.tile([128, 1], fp32, tag="rec", bufs=8)
                            nc.vector.reciprocal(rec, patb[:, DK : DK + 1])
                            nc.vector.tensor_scalar_mul(
                                att_t[:, s, hs : hs + DK],
                                in0=patb[:, 0:DK],
                                scalar1=rec,
                            )
                    # broadcast add + store for this q-group's rows (one DMA
                    # per batch: every attempt to split these across more
                    # queues lost more to per-DMA issue cost than it gained
                    # in transfer parallelism)
                    for b in range(B):
                        nc.vector.tensor_add(
                            out=tgtc_sb[:, b, qg * 4 : (qg + 1) * 4, :],
                            in0=tgtc_sb[:, b, qg * 4 : (qg + 1) * 4, :],
                            in1=att_t,
                        )
                        nc.sync.dma_start(
                            out=outc[b, :, qg * 4 : (qg + 1) * 4, :],
                            in_=tgtc_sb[:, b, qg * 4 : (qg + 1) * 4, :],
                        )

    nc.finalize()
    return nc


def _get_nc(n_rows):
    if n_rows not in _CACHE:
        _CACHE[n_rows] = _build(n_rows)
    return _CACHE[n_rows]


def _bf16(x):
    return np.ascontiguousarray(x, dtype=np.float32).astype(ml_dtypes.bfloat16)


def _run(tgt, memory, Wq, Wk, Wv, trace=False):
    global LAST_RESULTS
    from concourse.bass_utils import run_bass_kernel_spmd

    n_rows = tgt.shape[1]
    G = n_rows // 512
    RT = n_rows // 128
    nc = _get_nc(n_rows)

    tgt = np.ascontiguousarray(tgt, dtype=np.float32)
    memory = np.ascontiguousarray(memory, dtype=np.float32)

    def grouped(x0):  # [N, D] -> [G, 128, DC, 512] with [g,p,d,n] = x0[g*512+n, d*128+p]
        return _bf16(
            np.ascontiguousarray(
                x0.reshape(G, 512, DC, 128).transpose(0, 3, 2, 1)
            )
        )

    mem0g = grouped(memory[0])
    tgt0g = grouped(tgt[0])

    def wslice(W, sl):  # [D_out, D_in] slice -> [128, DC, CW]
        return _bf16(
            np.ascontiguousarray(
                W[sl, :].T.reshape(DC, 128, CW).transpose(1, 0, 2)
            )
        )

    in_maps = []
    for c in range(NCORES):
        sl = slice(c * CW, (c + 1) * CW)
        tc_arr = _bf16(
            np.ascontiguousarray(
                tgt[:, :, sl].reshape(B, RT, 128, CW).transpose(0, 2, 1, 3)
            )
        )
        in_maps.append(
            {
                "mem0g": mem0g,
                "tgt0g": tgt0g,
                "wqt": wslice(Wq, sl),
                "wkt": wslice(Wk, sl),
                "wvt": wslice(Wv, sl),
                "tgtc": tc_arr,
            }
        )
    res = run_bass_kernel_spmd(nc, in_maps, list(range(NCORES)), trace=trace)
    LAST_RESULTS = res
    out = np.empty((B, n_rows, NCORES * CW), dtype=np.float32)
    for c in range(NCORES):
        oc = np.asarray(res.results[c]["outc"], dtype=np.float32)
        # [B, 128, RT, CW] -> [B, N, CW]
        out[:, :, c * CW : (c + 1) * CW] = oc.transpose(0, 2, 1, 3).reshape(
            B, n_rows, CW
        )
    return out


def kernel(tgt, memory, Wq, Wk, Wv):
    return _run(tgt, memory, Wq, Wk, Wv)


# revision 56
# speedup vs baseline: 1.0275x; 1.0020x over previous
import sys

sys.path.insert(0, "/opt/trn_rl_repo")

import numpy as np
import ml_dtypes

NCORES = 8
B, FULL_N, D = 4, 2048, 1024
NH = 16
DK = 64  # head dim
HPC = NH // NCORES  # heads per core = 2
CW = HPC * DK  # output columns per core = 128
DC = D // 128  # D chunks = 8

_CACHE = {}
LAST_RESULTS = None


def _build(n_rows):
    """SPMD Bass program for one core. Each core computes batch-0 attention
    for its 2 heads (the reference only uses att[0]) and adds it to its
    column slice of tgt for all batches.

    All-bf16 datapath: tgt[0]/memory[0]/weights arrive host-transposed and
    pre-packed in bf16 (halves HBM traffic vs fp32, and bf16 keeps the PE
    at full HAM duty for ~90us where fp32r/fp8-DoubleRow configs get
    duty-cycle throttled). Scores are computed transposed (k on partitions)
    so softmax's P feeds P.T@V with no P transposes. exp() runs on ScalarE
    with scale 1/sqrt(dk) and bias -4 (cancels exactly in the softmax
    ratio), writing P as bf16; V carries an appended ones column so the PV
    accumulation also yields the softmax row sums. Projections/scores/PV
    are software-pipelined across 512-row q-groups (scores of qg+1 overlap
    PV of qg) so neither PE nor ScalarE ever starves; input group DMAs are
    triple-buffered so all loads stream from t=0."""
    import concourse.mybir as mybir
    import concourse.tile as tile
    from concourse import bacc
    from concourse.masks import make_identity

    fp32 = mybir.dt.float32
    bf16 = mybir.dt.bfloat16
    fp8 = mybir.dt.float8e4

    RT = n_rows // 128  # row tiles
    G = n_rows // 512  # 512-row groups
    QG = G
    KC = RT  # k chunks of 128

    nc = bacc.Bacc(None, target_bir_lowering=False)
    # host pre-arranged layouts (fully linear per-partition DMA):
    #   mem0g/tgt0g[g, p, d, n] = x[g*512 + n, d*128 + p]   (x = memory[0]/tgt[0])
    #   w*[p, d, q]             = W[c*CW + q, d*128 + p]     (core c's slice)
    #   tgtc[b, p, t, c]        = tgt[b, t*128 + p, c0 + c]
    mem0g = nc.declare_dram_parameter("mem0g", [G, 128, DC, 512], bf16, isOutput=False)
    tgt0g = nc.declare_dram_parameter("tgt0g", [G, 128, DC, 512], bf16, isOutput=False)
    wqt = nc.declare_dram_parameter("wqt", [128, DC, CW], bf16, isOutput=False)
    wkt = nc.declare_dram_parameter("wkt", [128, DC, CW], bf16, isOutput=False)
    wvt = nc.declare_dram_parameter("wvt", [128, DC, CW], bf16, isOutput=False)
    tgtc = nc.declare_dram_parameter("tgtc", [B, 128, RT, CW], bf16, isOutput=False)
    outc = nc.declare_dram_parameter("outc", [B, 128, RT, CW], bf16, isOutput=True)

    Exp = mybir.ActivationFunctionType.Exp
    DR = mybir.MatmulPerfMode.DoubleRow
    scale = 1.0 / np.sqrt(DK)
    EBIAS = -4.0  # exp(s*scale + EBIAS): keeps p in fp8e4m3 range; cancels in ratio

    with tile.TileContext(nc) as tc:
        # single SBUF + single PSUM pool (tags carry the per-buffer cycling):
        # every tile pool costs an all-engine drain round at release, ~8us of
        # teardown barriers with 13 pools
        with (
            tc.tile_pool(name="sbp", bufs=1) as const,
            tc.tile_pool(name="psp", bufs=2, space="PSUM") as psp,
        ):
            persist = const
            # PE warmup on a never-written tile: no dependency on
            # make_identity/gpsimd preamble, so the p-state ramp starts at
            # ~0.2us instead of ~7us (values are discarded)
            garb = const.tile([128, 128], bf16, tag="garb")
            nc.vector.memset(garb, 0.0)
            for _ in range(16):
                pw = psp.tile([128, 128], bf16, tag="pj", name="warm")
                nc.tensor.transpose(pw, garb, garb)
            ident = const.tile([128, 128], bf16, tag="ident")
            make_identity(nc, ident)
            ebias = const.tile([128, 1], fp32, tag="ebias")
            nc.vector.memset(ebias, EBIAS)

            KT_gs = [
                persist.tile([128, 512], bf16, tag=f"KT{g}", name=f"KT{g}")
                for g in range(G)
            ]
            QT_gs = [
                persist.tile([128, 512], bf16, tag=f"QT{g}", name=f"QT{g}")
                for g in range(G)
            ]
            # Vp[g][p, h, t, :] = V chunk (4g + t) for head h:
            # rows = k-positions, cols = dk dims + ones column
            VW = DK + 1
            Vp_gs = [
                persist.tile(
                    [128, HPC, 4, VW], bf16, tag=f"Vp{g}", name=f"Vp{g}"
                )
                for g in range(G)
            ]
            for g in range(G):
                nc.vector.memset(Vp_gs[g][:, :, :, DK], 1.0)
            tgtc_sb = persist.tile([128, B, RT, CW], bf16, tag="tgtc")

            wst_pool = mgrp_pool = tgrp_pool = vt_pool = const
            usb_pool = att_pool = small_pool = pt_pool = const
            ps_pj = ps_sc = ps_pv = psp
            if True:
                WTs = {}
                for name, w in (("k", wkt), ("v", wvt), ("q", wqt)):
                    wt = wst_pool.tile([128, DC, CW], bf16, tag=f"wt{name}")
                    nc.sync.dma_start(out=wt, in_=w[:, :, :])
                    WTs[name] = wt

                def emit_K(g):
                    memg = mgrp_pool.tile(
                        [128, DC, 512], bf16, tag="mg", bufs=3, name=f"memg{g}"
                    )
                    for d in range(DC):
                        nc.sync.dma_start(out=memg[:, d, :], in_=mem0g[g, :, d, :])
                    pk = ps_pj.tile([128, 512], fp32, tag="pj", name=f"psk{g}")
                    for d in range(DC):
                        nc.tensor.matmul(
                            pk, WTs["k"][:, d, :], memg[:, d, :],
                            start=(d == 0), stop=(d == DC - 1),
                        )
                    nc.vector.tensor_copy(out=KT_gs[g], in_=pk)
                    return memg

                def emit_V(g, memg):
                    pv = ps_pj.tile([128, 512], fp32, tag="pj", name=f"psv{g}")
                    for d in range(DC):
                        nc.tensor.matmul(
                            pv, WTs["v"][:, d, :], memg[:, d, :],
                            start=(d == 0), stop=(d == DC - 1),
                        )
                    vt_g = vt_pool.tile([128, 512], bf16, tag="vtg", bufs=2, name=f"vt{g}")
                    nc.vector.tensor_copy(out=vt_g, in_=pv)
                    for t in range(4):
                        ptb = ps_pj.tile([128, 128], bf16, tag="pj", name=f"vtr{g}_{t}")
                        nc.tensor.transpose(
                            ptb, vt_g[:, t * 128 : (t + 1) * 128], ident
                        )
                        for h in range(HPC):
                            nc.vector.tensor_copy(
                                out=Vp_gs[g][:, h, t, 0:DK],
                                in_=ptb[:, h * DK : (h + 1) * DK],
                            )

                def emit_tgt_group(g):
                    tgtg = tgrp_pool.tile(
                        [128, DC, 512], bf16, tag="tg", bufs=3, name=f"tgtg{g}"
                    )
                    for d in range(DC):
                        nc.sync.dma_start(out=tgtg[:, d, :], in_=tgt0g[g, :, d, :])
                    pq = ps_pj.tile([128, 512], fp32, tag="pj", name=f"psq{g}")
                    for d in range(DC):
                        nc.tensor.matmul(
                            pq, WTs["q"][:, d, :], tgtg[:, d, :],
                            start=(d == 0), stop=(d == DC - 1),
                        )
                    nc.vector.tensor_copy(out=QT_gs[g], in_=pq)

                def emit_st_block(qg, pts, jp):
                    # one score+exp pair: k-chunks 2*jp, 2*jp+1 for both heads
                    psts = [
                        ps_sc.tile(
                            [128, 2, 512], fp32, tag="st", name=f"st{qg}_{jp}_{h}"
                        )
                        for h in range(HPC)
                    ]
                    for jj in range(2):
                        j = jp * 2 + jj
                        kg, kt = j // 4, j % 4
                        for h in range(HPC):
                            hs = h * DK
                            nc.tensor.matmul(
                                psts[h][:, jj, :],
                                KT_gs[kg][hs : hs + DK, kt * 128 : (kt + 1) * 128],
                                QT_gs[qg][hs : hs + DK, :],
                                start=True, stop=True,
                            )
                    for h in range(HPC):
                        nc.scalar.activation(
                            out=pts[h][:, jp * 2 : jp * 2 + 2, :],
                            in_=psts[h],
                            func=Exp,
                            scale=float(scale),
                            bias=ebias,
                        )

                def new_pts(qg):
                    return [
                        pt_pool.tile(
                            [128, KC, 512], bf16, tag=f"pt{h}", bufs=2,
                            name=f"pt{h}_{qg}",
                        )
                        for h in range(HPC)
                    ]

                all_pts = {}
                # ---- Phase A: K0 -> Q0 -> first scores BEFORE any V work, so
                # ScalarE's exp stream (83% dense once started, and it paces
                # the whole back half of the kernel) starts ~8us earlier; V
                # projections and transposes backfill between score blocks.
                all_pts[0] = new_pts(0)
                memgs = {}
                # group 0 special-cased: mem/tgt chunk DMAs interleave on the
                # SP issue queue so Q-proj's inputs land alongside K-proj's
                # instead of ~7us later
                memg0 = mgrp_pool.tile(
                    [128, DC, 512], bf16, tag="mg", bufs=3, name="memg0"
                )
                tgtg0 = tgrp_pool.tile(
                    [128, DC, 512], bf16, tag="tg", bufs=3, name="tgtg0"
                )
                for d in range(DC):
                    nc.sync.dma_start(out=memg0[:, d, :], in_=mem0g[0, :, d, :])
                    nc.sync.dma_start(out=tgtg0[:, d, :], in_=tgt0g[0, :, d, :])
                pk0 = ps_pj.tile([128, 512], fp32, tag="pj", name="psk0")
                for d in range(DC):
                    nc.tensor.matmul(
                        pk0, WTs["k"][:, d, :], memg0[:, d, :],
                        start=(d == 0), stop=(d == DC - 1),
                    )
                nc.vector.tensor_copy(out=KT_gs[0], in_=pk0)
                pq0 = ps_pj.tile([128, 512], fp32, tag="pj", name="psq0")
                for d in range(DC):
                    nc.tensor.matmul(
                        pq0, WTs["q"][:, d, :], tgtg0[:, d, :],
                        start=(d == 0), stop=(d == DC - 1),
                    )
                nc.vector.tensor_copy(out=QT_gs[0], in_=pq0)
                memgs[0] = memg0
                for jp in (0, 1):
                    emit_st_block(0, all_pts[0], jp)
                emit_V(0, memgs[0])
                memgs[1] = emit_K(1)
                for jp in (2, 3):
                    emit_st_block(0, all_pts[0], jp)
                emit_V(1, memgs[1])
                emit_tgt_group(1)
                for g in range(2, G):
                    memgs[g] = emit_K(g)
                    for jp in range(2 * g, 2 * g + 2):
                        emit_st_block(0, all_pts[0], jp)
                    emit_V(g, memgs[g])
                # residual input, only needed for the adds at each q-group tail
                for b in range(B):
                    nc.sync.dma_start(out=tgtc_sb[:, b, :, :], in_=tgtc[b])

                # ---- Phase B: per q-group: scores(qg+1) overlap PV(qg) ----
                for qg in range(QG):
                    if qg + 2 < QG:
                        emit_tgt_group(qg + 2)
                    if qg + 1 < QG:
                        all_pts[qg + 1] = new_pts(qg + 1)
                        for jp in range(KC // 2):
                            emit_st_block(qg + 1, all_pts[qg + 1], jp)
                    pts = all_pts[qg]
                    att_t = att_pool.tile(
                        [128, 4, CW], bf16, tag="att", bufs=2, name=f"att{qg}"
                    )
                    # both heads' PV accumulations first; the pu->SBUF copies
                    # then hide under the other head's matmuls so the att
                    # transposes never stall the PE on a DVE copy
                    pu_sbs = {}
                    for h in range(HPC):
                        pu = ps_pv.tile([VW, 512], fp32, tag="u", name=f"u{qg}{h}")
                        for j in range(KC):
                            nc.tensor.matmul(
                                pu,
                                Vp_gs[j // 4][:, h, j % 4, :],
                                pts[h][:, j, :],
                                start=(j == 0), stop=(j == KC - 1),
                            )
                        pu_sb = usb_pool.tile([DK + 1, 512], bf16, tag="usb", bufs=2)
                        nc.vector.tensor_copy(out=pu_sb, in_=pu[0 : DK + 1, :])
                        pu_sbs[h] = pu_sb
                    for h in range(HPC):
                        hs = h * DK
                        for s in range(4):
                            patb = ps_pj.tile(
                                [128, DK + 1], bf16, tag="pj", name=f"atr{qg}{h}{s}"
                            )
                            nc.tensor.transpose(
                                patb,
                                pu_sbs[h][:, s * 128 : (s + 1) * 128],
                                ident[0 : DK + 1, 0 : DK + 1],
                            )
                            rec = small_pool.tile([128, 1], fp32, tag="rec", bufs=8)
                            nc.vector.reciprocal(rec, patb[:, DK : DK + 1])
                            nc.vector.tensor_scalar_mul(
                                att_t[:, s, hs : hs + DK],
                                in0=patb[:, 0:DK],
                                scalar1=rec,
                            )
                    # broadcast add + store for this q-group's rows (one DMA
                    # per batch: every attempt to split these across more
                    # queues lost more to per-DMA issue cost than it gained
                    # in transfer parallelism)
                    for b in range(B):
                        nc.vector.tensor_add(
                            out=tgtc_sb[:, b, qg * 4 : (qg + 1) * 4, :],
                            in0=tgtc_sb[:, b, qg * 4 : (qg + 1) * 4, :],
                            in1=att_t,
                        )
                        nc.sync.dma_start(
                            out=outc[b, :, qg * 4 : (qg + 1) * 4, :],
                            in_=tgtc_sb[:, b, qg * 4 : (qg + 1) * 4, :],
                        )

    nc.finalize()
    return nc


def _get_nc(n_rows):
    if n_rows not in _CACHE:
        _CACHE[n_rows] = _build(n_rows)
    return _CACHE[n_rows]


def _bf16(x):
    return np.ascontiguousarray(x, dtype=np.float32).astype(ml_dtypes.bfloat16)


def _run(tgt, memory, Wq, Wk, Wv, trace=False):
    global LAST_RESULTS
    from concourse.bass_utils import run_bass_kernel_spmd

    n_rows = tgt.shape[1]
    G = n_rows // 512
    RT = n_rows // 128
    nc = _get_nc(n_rows)

    tgt = np.ascontiguousarray(tgt, dtype=np.float32)
    memory = np.ascontiguousarray(memory, dtype=np.float32)

    def grouped(x0):  # [N, D] -> [G, 128, DC, 512] with [g,p,d,n] = x0[g*512+n, d*128+p]
        return _bf16(
            np.ascontiguousarray(
                x0.reshape(G, 512, DC, 128).transpose(0, 3, 2, 1)
            )
        )

    mem0g = grouped(memory[0])
    tgt0g = grouped(tgt[0])

    def wslice(W, sl):  # [D_out, D_in] slice -> [128, DC, CW]
        return _bf16(
            np.ascontiguousarray(
                W[sl, :].T.reshape(DC, 128, CW).transpose(1, 0, 2)
            )
        )

    in_maps = []
    for c in range(NCORES):
        sl = slice(c * CW, (c + 1) * CW)
        tc_arr = _bf16(
            np.ascontiguousarray(
                tgt[:, :, sl].reshape(B, RT, 128, CW).transpose(0, 2, 1, 3)
            )
        )
        in_maps.append(
            {
                "mem0g": mem0g,
                "tgt0g": tgt0g,
                "wqt": wslice(Wq, sl),
                "wkt": wslice(Wk, sl),
                "wvt": wslice(Wv, sl),
                "tgtc": tc_arr,
            }
        )
    res = run_bass_kernel_spmd(nc, in_maps, list(range(NCORES)), trace=trace)
    LAST_RESULTS = res
    out = np.empty((B, n_rows, NCORES * CW), dtype=np.float32)
    for c in range(NCORES):
        oc = np.asarray(res.results[c]["outc"], dtype=np.float32)
        # [B, 128, RT, CW] -> [B, N, CW]
        out[:, :, c * CW : (c + 1) * CW] = oc.transpose(0, 2, 1, 3).reshape(
            B, n_rows, CW
        )
    return out


def kernel(tgt, memory, Wq, Wk, Wv):
    return _run(tgt, memory, Wq, Wk, Wv)
